# revision 1
# baseline (speedup 1.0000x reference)
"""Positional-encoding add for X (8192, 4096) f32 on 8 TRN2 NeuronCores.

out[p, j] = X[p, j] + (sin(p * w_j) if j even else cos(p * w_j'))
with w_j = 10000**(-(j + j%2)/n).

Strategy: shard rows across 8 cores (1024 rows each). Each core streams
its X shard through SBUF (8 tiles of 128x4096) and computes the
positional table on the fly:
  t  = pos * w/(2pi)            per-partition scalar multiply (+0.25 on
                                odd cols: cos(x) = sin(x + pi/2))
  k  = (t + 1.5*2^23) - 1.5*2^23   RNE round-to-int via magic number
  f  = t - k            in [-0.5, 0.5], exact (Sterbenz)
  s  = Sin(2*pi*f)      ACT engine (its Sin has no range reduction,
                        hence the reduction above)
  out = x + s
DMA per core = 16MB in + 16MB out + 2MB table consts: ~memory roofline.
"""
import sys

sys.path.insert(0, "/opt/trn_rl_repo")
import numpy as np

M_ROWS, N_COLS = 8192, 4096
N_CORES = 8
ROWS_PER_CORE = M_ROWS // N_CORES      # 1024
P = 128
TILES_PER_CORE = ROWS_PER_CORE // P    # 8
MAGIC = float(np.float32(1.5 * 2**23))
TWO_PI = float(2.0 * np.pi)

_nc = None
_host_consts = None


def _build_nc():
    global _nc
    if _nc is not None:
        return _nc
    import concourse.bacc as bacc
    import concourse.mybir as mybir
    from concourse.tile import TileContext

    dt = mybir.dt.float32
    Alu = mybir.AluOpType

    nc = bacc.Bacc("TRN2", target_bir_lowering=False, debug=False)
    x = nc.dram_tensor("x", [ROWS_PER_CORE, N_COLS], dt, kind="ExternalInput")
    w = nc.dram_tensor("w2pi", [P, N_COLS], dt, kind="ExternalInput")
    pos = nc.dram_tensor("pos", [P, TILES_PER_CORE], dt, kind="ExternalInput")
    out = nc.dram_tensor("out", [ROWS_PER_CORE, N_COLS], dt, kind="ExternalOutput")

    with TileContext(nc) as tc:
        with (
            tc.tile_pool(name="const", bufs=1) as cpool,
            tc.tile_pool(name="xp", bufs=3) as xpool,
            tc.tile_pool(name="fp", bufs=2) as fpool,
            tc.tile_pool(name="kp", bufs=2) as kpool,
        ):
            tw = cpool.tile([P, N_COLS], dt)
            tpos = cpool.tile([P, TILES_PER_CORE], dt)
            nc.sync.dma_start(out=tw[:], in_=w[:])
            nc.sync.dma_start(out=tpos[:], in_=pos[:])
            for t in range(TILES_PER_CORE):
                r0 = t * P
                tx = xpool.tile([P, N_COLS], dt)
                nc.sync.dma_start(out=tx[:], in_=x[r0:r0 + P, :])
                tf = fpool.tile([P, N_COLS], dt)
                ps = tpos[:, t:t + 1]
                nc.vector.tensor_scalar(
                    out=tf[:, 0::2], in0=tw[:, 0::2],
                    scalar1=ps, scalar2=None, op0=Alu.mult)
                nc.vector.tensor_scalar(
                    out=tf[:, 1::2], in0=tw[:, 1::2],
                    scalar1=ps, scalar2=0.25, op0=Alu.mult, op1=Alu.add)
                tk = kpool.tile([P, N_COLS], dt)
                nc.gpsimd.tensor_scalar(
                    out=tk[:], in0=tf[:],
                    scalar1=MAGIC, scalar2=MAGIC, op0=Alu.add, op1=Alu.subtract)
                nc.vector.tensor_tensor(
                    out=tf[:], in0=tf[:], in1=tk[:], op=Alu.subtract)
                nc.scalar.activation(
                    tk[:], tf[:], mybir.ActivationFunctionType.Sin, scale=TWO_PI)
                nc.vector.tensor_tensor(
                    out=tx[:], in0=tx[:], in1=tk[:], op=Alu.add)
                nc.sync.dma_start(out=out[r0:r0 + P, :], in_=tx[:])
    nc.compile()
    _nc = nc
    return nc


def _consts():
    """w/(2pi) broadcast row and per-core position columns."""
    global _host_consts
    if _host_consts is not None:
        return _host_consts
    j = np.arange(N_COLS)
    e = ((j + (j % 2)) / N_COLS).astype(np.float32)
    # match the reference's f32 10000**e as closely as possible
    try:
        import jax
        import jax.numpy as jnp
        cpu = jax.devices("cpu")[0]
        with jax.default_device(cpu):
            denom = np.asarray(
                jnp.power(jnp.float32(10000.0), jnp.asarray(e, jnp.float32)))
    except Exception:
        denom = np.power(np.float32(10000.0), e).astype(np.float32)
    w2pi_row = (1.0 / (2.0 * np.pi * denom.astype(np.float64))).astype(np.float32)
    W2PI = np.ascontiguousarray(np.broadcast_to(w2pi_row, (P, N_COLS)))
    q = np.arange(P, dtype=np.float32)[:, None]
    t = np.arange(TILES_PER_CORE, dtype=np.float32)[None, :]
    POS = [
        np.ascontiguousarray(c * ROWS_PER_CORE + t * P + q, dtype=np.float32)
        for c in range(N_CORES)
    ]
    _host_consts = (W2PI, POS)
    return _host_consts


def _run(X, trace=False):
    from concourse.bass_utils import run_bass_kernel_spmd

    nc = _build_nc()
    W2PI, POS = _consts()
    X = np.ascontiguousarray(np.asarray(X, dtype=np.float32))
    assert X.shape == (M_ROWS, N_COLS)
    in_maps = [
        {
            "x": np.ascontiguousarray(
                X[c * ROWS_PER_CORE:(c + 1) * ROWS_PER_CORE]),
            "w2pi": W2PI,
            "pos": POS[c],
        }
        for c in range(N_CORES)
    ]
    res = run_bass_kernel_spmd(
        nc, in_maps, core_ids=list(range(N_CORES)), trace=trace)
    full = np.concatenate(
        [res.results[c]["out"] for c in range(N_CORES)], axis=0)
    return full, res


def kernel(X):
    full, _ = _run(X, trace=False)
    return full


def run_traced(X):
    """For test.py: returns (output, BassKernelResults with exec_time_ns)."""
    return _run(X, trace=True)


# revision 4
# speedup vs baseline: 2.2204x; 2.2204x over previous
"""Positional-encoding add for X (8192, 4096) f32 on 8 TRN2 NeuronCores.

out[p, j] = X[p, j] + (sin(p * w_j) if j even else cos(p * w_j'))
with w_j = 10000**(-(j + j%2)/n).

Strategy: shard rows across 8 cores (1024 rows each). Each core streams
its X shard through SBUF (8 tiles of 128x4096) and computes the
positional table on the fly:
  t  = pos * w/(2pi)            per-partition scalar multiply (+0.25 on
                                odd cols: cos(x) = sin(x + pi/2))
  k  = (t + 1.5*2^23) - 1.5*2^23   RNE round-to-int via magic number
  f  = t - k            in [-0.5, 0.5], exact (Sterbenz)
  s  = Sin(2*pi*f)      ACT engine (its Sin has no range reduction,
                        hence the reduction above)
  out = x + s
DMA per core = 16MB in + 16MB out + 2MB table consts: ~memory roofline.
"""
import sys

sys.path.insert(0, "/opt/trn_rl_repo")
import numpy as np

M_ROWS, N_COLS = 8192, 4096
N_CORES = 8
ROWS_PER_CORE = M_ROWS // N_CORES      # 1024
P = 128
TILES_PER_CORE = ROWS_PER_CORE // P    # 8
MAGIC = float(np.float32(1.5 * 2**23))
TWO_PI = float(2.0 * np.pi)

_nc_cache = {}
_host_consts = None


def _build_nc(reps=1):
    if reps in _nc_cache:
        return _nc_cache[reps]
    import concourse.bacc as bacc
    import concourse.mybir as mybir
    from concourse.tile import TileContext

    dt = mybir.dt.float32
    Alu = mybir.AluOpType

    nc = bacc.Bacc("TRN2", target_bir_lowering=False, debug=False)
    x = nc.dram_tensor("x", [ROWS_PER_CORE, N_COLS], dt, kind="ExternalInput")
    w = nc.dram_tensor("w2pi", [P, N_COLS], dt, kind="ExternalInput")
    pos = nc.dram_tensor("pos", [P, TILES_PER_CORE], dt, kind="ExternalInput")
    out = nc.dram_tensor("out", [ROWS_PER_CORE, N_COLS], dt, kind="ExternalOutput")

    with TileContext(nc) as tc:
        with (
            tc.tile_pool(name="const", bufs=1) as cpool,
            tc.tile_pool(name="xp", bufs=3) as xpool,
            tc.tile_pool(name="fp", bufs=2) as fpool,
            tc.tile_pool(name="kp", bufs=2) as kpool,
        ):
            tw = cpool.tile([P, N_COLS], dt)
            tpos = cpool.tile([P, TILES_PER_CORE], dt)
            nc.sync.dma_start(out=tw[:], in_=w[:])
            nc.sync.dma_start(out=tpos[:], in_=pos[:])
            for t in range(TILES_PER_CORE * reps):
                t = t % TILES_PER_CORE
                r0 = t * P
                tx = xpool.tile([P, N_COLS], dt)
                nc.sync.dma_start(out=tx[:], in_=x[r0:r0 + P, :])
                tf = fpool.tile([P, N_COLS], dt)
                ps = tpos[:, t:t + 1]
                nc.vector.tensor_scalar(
                    out=tf[:, 0::2], in0=tw[:, 0::2],
                    scalar1=ps, scalar2=None, op0=Alu.mult)
                nc.vector.tensor_scalar(
                    out=tf[:, 1::2], in0=tw[:, 1::2],
                    scalar1=ps, scalar2=0.25, op0=Alu.mult, op1=Alu.add)
                tk = kpool.tile([P, N_COLS], dt)
                nc.gpsimd.tensor_scalar(
                    out=tk[:], in0=tf[:],
                    scalar1=MAGIC, scalar2=MAGIC, op0=Alu.add, op1=Alu.subtract)
                nc.vector.tensor_tensor(
                    out=tf[:], in0=tf[:], in1=tk[:], op=Alu.subtract)
                nc.scalar.activation(
                    tk[:], tf[:], mybir.ActivationFunctionType.Sin, scale=TWO_PI)
                nc.vector.tensor_tensor(
                    out=tx[:], in0=tx[:], in1=tk[:], op=Alu.add)
                nc.sync.dma_start(out=out[r0:r0 + P, :], in_=tx[:])
    nc.compile()
    _nc_cache[reps] = nc
    return nc


def _consts():
    """w/(2pi) broadcast row and per-core position columns."""
    global _host_consts
    if _host_consts is not None:
        return _host_consts
    j = np.arange(N_COLS)
    e = ((j + (j % 2)) / N_COLS).astype(np.float32)
    # match the reference's f32 10000**e as closely as possible
    try:
        import jax
        import jax.numpy as jnp
        cpu = jax.devices("cpu")[0]
        with jax.default_device(cpu):
            denom = np.asarray(
                jnp.power(jnp.float32(10000.0), jnp.asarray(e, jnp.float32)))
    except Exception:
        denom = np.power(np.float32(10000.0), e).astype(np.float32)
    w2pi_row = (1.0 / (2.0 * np.pi * denom.astype(np.float64))).astype(np.float32)
    W2PI = np.ascontiguousarray(np.broadcast_to(w2pi_row, (P, N_COLS)))
    q = np.arange(P, dtype=np.float32)[:, None]
    t = np.arange(TILES_PER_CORE, dtype=np.float32)[None, :]
    POS = [
        np.ascontiguousarray(c * ROWS_PER_CORE + t * P + q, dtype=np.float32)
        for c in range(N_CORES)
    ]
    _host_consts = (W2PI, POS)
    return _host_consts


def _run(X, trace=False):
    from concourse.bass_utils import run_bass_kernel_spmd

    nc = _build_nc()
    W2PI, POS = _consts()
    X = np.ascontiguousarray(np.asarray(X, dtype=np.float32))
    assert X.shape == (M_ROWS, N_COLS)
    in_maps = [
        {
            "x": np.ascontiguousarray(
                X[c * ROWS_PER_CORE:(c + 1) * ROWS_PER_CORE]),
            "w2pi": W2PI,
            "pos": POS[c],
        }
        for c in range(N_CORES)
    ]
    res = run_bass_kernel_spmd(
        nc, in_maps, core_ids=list(range(N_CORES)), trace=trace)
    full = np.concatenate(
        [res.results[c]["out"] for c in range(N_CORES)], axis=0)
    return full, res


def kernel(X):
    full, _ = _run(X, trace=False)
    return full


def run_traced(X):
    """For test.py: returns (output, BassKernelResults with exec_time_ns)."""
    return _run(X, trace=True)


# revision 25
# speedup vs baseline: 8.2412x; 3.7116x over previous
"""Positional-encoding add for X (8192, 4096) f32 on 8 TRN2 NeuronCores.

out[p, j] = X[p, j] + (sin(p * w_j) if j even else cos(p * w_j'))
with w_j = 10000**(-(j + j%2)/n).

Strategy: shard rows across 8 cores (1024 rows each). Each core streams
its X shard through SBUF (8 tiles of 128x4096) and computes the
positional table on the fly (production variant "v3"), in cycles
(t = angle/2pi) so range reduction is a round-to-int:
  ACT: t0 = pos * w/(2pi)       (Copy with per-partition scale)
  DVE: t  = t0 + phase          (phase = 0.25 on odd cols:
                                 cos(x) = sin(x + pi/2))
  ACT: c1 = t + 1.5*2^23 ; k = c1 - 1.5*2^23   (RNE round via magic)
  DVE: f  = t - k               in [-0.5, 0.5], exact (Sterbenz)
  ACT: s  = Sin(2*pi*f)         (ACT Sin has no range reduction --
                                 garbage outside ~[-pi, pi])
  DVE: out = x + s

Engine choice is driven by measured TRN2 behavior: stride-2 DVE ops run
~19x slower than unit-stride (so no even/odd strided ops anywhere),
GpSimd tensor ops are ~15x slower than DVE (so nothing on GpSimd), ACT
runs at ~1 el/cycle/partition. All DVE/ACT ops are full-width
unit-stride; work splits 3 DVE / 4 ACT ops per tile, both under the
DMA roofline. DMA per core = 16MB in + 16MB out + 4MB consts:
~memory-bound at ~358 GB/s/core (~100 us/core).
"""
import sys

sys.path.insert(0, "/opt/trn_rl_repo")
import numpy as np

M_ROWS, N_COLS = 8192, 4096
N_CORES = 8
ROWS_PER_CORE = M_ROWS // N_CORES      # 1024
P = 128
TILES_PER_CORE = ROWS_PER_CORE // P    # 8
MAGIC = float(np.float32(1.5 * 2**23))
TWO_PI = float(2.0 * np.pi)

_nc_cache = {}
_host_consts = None


def _build_nc(mult=1, variant="full", bufs=(3, 2, 2)):
    """mult: row multiplier for benchmarking (shard of mult*1024 rows).
    The production kernel uses mult=1. bufs = (x, f, k) pool depths."""
    key = (mult, variant, bufs)
    if key in _nc_cache:
        return _nc_cache[key]
    import concourse.bacc as bacc
    import concourse.mybir as mybir
    from concourse.tile import TileContext

    dt = mybir.dt.float32
    Alu = mybir.AluOpType
    n_rows = ROWS_PER_CORE * mult
    n_tiles = TILES_PER_CORE * mult

    nc = bacc.Bacc("TRN2", target_bir_lowering=False, debug=False)
    x = nc.dram_tensor("x", [n_rows, N_COLS], dt, kind="ExternalInput")
    w = nc.dram_tensor("w2pi", [P, N_COLS], dt, kind="ExternalInput")
    pos = nc.dram_tensor("pos", [P, n_tiles], dt, kind="ExternalInput")
    phase = None
    if variant in ("v3", "v4"):
        phase = nc.dram_tensor("phase", [P, N_COLS], dt, kind="ExternalInput")
    out = nc.dram_tensor("out", [n_rows, N_COLS], dt, kind="ExternalOutput")

    with TileContext(nc) as tc:
        with (
            tc.tile_pool(name="const", bufs=1) as cpool,
            tc.tile_pool(name="xp", bufs=bufs[0]) as xpool,
            tc.tile_pool(name="fp", bufs=bufs[1]) as fpool,
            tc.tile_pool(name="kp", bufs=bufs[2]) as kpool,
        ):
            tw = cpool.tile([P, N_COLS], dt)
            tpos = cpool.tile([P, n_tiles], dt)
            nc.sync.dma_start(out=tw[:], in_=w[:])
            nc.sync.dma_start(out=tpos[:], in_=pos[:])
            tph = None
            if phase is not None:
                tph = cpool.tile([P, N_COLS], dt)
                nc.sync.dma_start(out=tph[:], in_=phase[:])
            for t in range(n_tiles):
                r0 = t * P
                tx = xpool.tile([P, N_COLS], dt)
                nc.sync.dma_start(out=tx[:], in_=x[r0:r0 + P, :])
                if variant == "dma":
                    nc.sync.dma_start(out=out[r0:r0 + P, :], in_=tx[:])
                    continue
                if variant in ("v3", "v4"):
                    # No strided ops (stride-2 DVE measured ~19x slow).
                    # ACT: t0 = pos*w' ; DVE: t = t0 + phase ; ACT: magic
                    # round pair ; DVE: f = t - k ; ACT: Sin ; DVE: add.
                    ps = tpos[:, t:t + 1]
                    tf = fpool.tile([P, N_COLS], dt)
                    tk = kpool.tile([P, N_COLS], dt)
                    if variant == "v3":
                        nc.scalar.activation(
                            tk[:], tw[:], mybir.ActivationFunctionType.Copy,
                            scale=ps)
                        nc.vector.tensor_tensor(
                            out=tf[:], in0=tk[:], in1=tph[:], op=Alu.add)
                    else:  # v4: t on DVE in one fused op? needs 2 ops
                        nc.vector.tensor_scalar(
                            out=tk[:], in0=tw[:],
                            scalar1=ps, scalar2=None, op0=Alu.mult)
                        nc.vector.tensor_tensor(
                            out=tf[:], in0=tk[:], in1=tph[:], op=Alu.add)
                    nc.scalar.activation(
                        tk[:], tf[:], mybir.ActivationFunctionType.Copy,
                        bias=MAGIC)
                    nc.scalar.activation(
                        tk[:], tk[:], mybir.ActivationFunctionType.Copy,
                        bias=-MAGIC)
                    nc.vector.tensor_tensor(
                        out=tf[:], in0=tf[:], in1=tk[:], op=Alu.subtract)
                    nc.scalar.activation(
                        tk[:], tf[:], mybir.ActivationFunctionType.Sin,
                        scale=TWO_PI)
                    nc.vector.tensor_tensor(
                        out=tx[:], in0=tx[:], in1=tk[:], op=Alu.add)
                    nc.sync.dma_start(out=out[r0:r0 + P, :], in_=tx[:])
                    continue
                tf = fpool.tile([P, N_COLS], dt)
                ps = tpos[:, t:t + 1]
                tk = kpool.tile([P, N_COLS], dt)
                if variant == "strideprobe":
                    # 24 extra strided ts pairs: measures stride-2 DVE rate
                    for _ in range(24):
                        nc.vector.tensor_scalar(
                            out=tf[:, 0::2], in0=tw[:, 0::2],
                            scalar1=ps, scalar2=None, op0=Alu.mult)
                        nc.vector.tensor_scalar(
                            out=tf[:, 1::2], in0=tw[:, 1::2],
                            scalar1=ps, scalar2=0.25,
                            op0=Alu.mult, op1=Alu.add)
                elif variant == "unitprobe":
                    # 24 extra unit-stride ts+tt pairs (2x element count of
                    # strideprobe pair): measures unit DVE rate
                    for _ in range(24):
                        nc.vector.tensor_scalar(
                            out=tf[:], in0=tw[:],
                            scalar1=ps, scalar2=None, op0=Alu.mult)
                        nc.vector.tensor_tensor(
                            out=tk[:], in0=tf[:], in1=tw[:], op=Alu.add)
                elif variant == "actprobe":
                    # 12 extra ACT sin ops: measures ACT rate
                    for _ in range(12):
                        nc.scalar.activation(
                            tk[:], tw[:],
                            mybir.ActivationFunctionType.Sin, scale=TWO_PI)
                elif variant == "unitprobe2":
                    # 24 interleaved independent-chain tt pairs: DVE
                    # streaming rate with depth-2 ILP
                    for _ in range(24):
                        nc.vector.tensor_tensor(
                            out=tf[:], in0=tw[:], in1=tx[:], op=Alu.add)
                        nc.vector.tensor_tensor(
                            out=tk[:], in0=tw[:], in1=tx[:], op=Alu.subtract)
                elif variant == "actcopyprobe":
                    # 24 ACT Copy ops (the magic-pair building block)
                    for _ in range(24):
                        nc.scalar.activation(
                            tk[:], tw[:], mybir.ActivationFunctionType.Copy,
                            bias=MAGIC)
                elif variant == "dvestream":
                    # DCE-proof DVE streaming probe: 4 round-robin chains
                    # (ILP=4), all results consumed into tx before DMA-out.
                    ring = [fpool.tile([P, N_COLS], dt, name=f"ring{i}",
                                       tag=f"ring{i}")
                            for i in range(4)]
                    for rb in ring:
                        nc.vector.tensor_tensor(
                            out=rb[:], in0=tw[:], in1=tx[:], op=Alu.add)
                    for _ in range(12):
                        for rb in ring:
                            nc.vector.tensor_tensor(
                                out=rb[:], in0=tw[:], in1=rb[:], op=Alu.add)
                    for rb in ring:
                        nc.vector.tensor_tensor(
                            out=tx[:], in0=tx[:], in1=rb[:], op=Alu.add)
                    nc.sync.dma_start(out=out[r0:r0 + P, :], in_=tx[:])
                    continue
                elif variant == "actstream":
                    # DCE-proof ACT streaming probe: 48 chained Copy ops
                    # consumed into tx. ACT Copy reads+writes same tile
                    # alternating two buffers for ILP.
                    ra = fpool.tile([P, N_COLS], dt)
                    rb2 = kpool.tile([P, N_COLS], dt)
                    nc.scalar.activation(
                        ra[:], tx[:], mybir.ActivationFunctionType.Copy,
                        bias=1.0)
                    nc.scalar.activation(
                        rb2[:], tx[:], mybir.ActivationFunctionType.Copy,
                        bias=2.0)
                    for _ in range(23):
                        nc.scalar.activation(
                            ra[:], ra[:], mybir.ActivationFunctionType.Copy,
                            bias=1.0)
                        nc.scalar.activation(
                            rb2[:], rb2[:], mybir.ActivationFunctionType.Copy,
                            bias=2.0)
                    nc.vector.tensor_tensor(
                        out=tx[:], in0=tx[:], in1=ra[:], op=Alu.add)
                    nc.vector.tensor_tensor(
                        out=tx[:], in0=tx[:], in1=rb2[:], op=Alu.add)
                    nc.sync.dma_start(out=out[r0:r0 + P, :], in_=tx[:])
                    continue
                if variant == "csplit":
                    # column-halved compute: shorter drain tail, DMAs stay 2MB
                    for h0 in (0, N_COLS // 2):
                        h1 = h0 + N_COLS // 2
                        nc.vector.tensor_scalar(
                            out=tf[:, h0:h1:2], in0=tw[:, h0:h1:2],
                            scalar1=ps, scalar2=None, op0=Alu.mult)
                        nc.vector.tensor_scalar(
                            out=tf[:, h0 + 1:h1:2], in0=tw[:, h0 + 1:h1:2],
                            scalar1=ps, scalar2=0.25,
                            op0=Alu.mult, op1=Alu.add)
                        nc.vector.tensor_scalar(
                            out=tk[:, h0:h1], in0=tf[:, h0:h1],
                            scalar1=MAGIC, scalar2=MAGIC,
                            op0=Alu.add, op1=Alu.subtract)
                        nc.vector.tensor_tensor(
                            out=tf[:, h0:h1], in0=tf[:, h0:h1],
                            in1=tk[:, h0:h1], op=Alu.subtract)
                        nc.scalar.activation(
                            tk[:, h0:h1], tf[:, h0:h1],
                            mybir.ActivationFunctionType.Sin, scale=TWO_PI)
                        nc.vector.tensor_tensor(
                            out=tx[:, h0:h1], in0=tx[:, h0:h1],
                            in1=tk[:, h0:h1], op=Alu.add)
                    nc.sync.dma_start(out=out[r0:r0 + P, :], in_=tx[:])
                    continue
                nc.vector.tensor_scalar(
                    out=tf[:, 0::2], in0=tw[:, 0::2],
                    scalar1=ps, scalar2=None, op0=Alu.mult)
                nc.vector.tensor_scalar(
                    out=tf[:, 1::2], in0=tw[:, 1::2],
                    scalar1=ps, scalar2=0.25, op0=Alu.mult, op1=Alu.add)
                if variant == "actmagic":
                    # ACT: c1 = t + M; k = c1 - M (both exact);
                    # DVE: f = t - k; ACT: Sin; DVE: add
                    nc.scalar.activation(
                        tk[:], tf[:], mybir.ActivationFunctionType.Copy,
                        bias=MAGIC)
                    nc.scalar.activation(
                        tk[:], tk[:], mybir.ActivationFunctionType.Copy,
                        bias=-MAGIC)
                elif variant == "gp":
                    nc.gpsimd.tensor_scalar(
                        out=tk[:], in0=tf[:],
                        scalar1=MAGIC, scalar2=MAGIC,
                        op0=Alu.add, op1=Alu.subtract)
                else:  # full: magic round on DVE
                    nc.vector.tensor_scalar(
                        out=tk[:], in0=tf[:],
                        scalar1=MAGIC, scalar2=MAGIC,
                        op0=Alu.add, op1=Alu.subtract)
                nc.vector.tensor_tensor(
                    out=tf[:], in0=tf[:], in1=tk[:], op=Alu.subtract)
                if variant == "nosin":
                    nc.vector.tensor_tensor(
                        out=tx[:], in0=tx[:], in1=tf[:], op=Alu.add)
                else:
                    nc.scalar.activation(
                        tk[:], tf[:], mybir.ActivationFunctionType.Sin,
                        scale=TWO_PI)
                    nc.vector.tensor_tensor(
                        out=tx[:], in0=tx[:], in1=tk[:], op=Alu.add)
                nc.sync.dma_start(out=out[r0:r0 + P, :], in_=tx[:])
    nc.compile()
    _nc_cache[key] = nc
    return nc


def _consts():
    """w/(2pi) broadcast row and per-core position columns."""
    global _host_consts
    if _host_consts is not None:
        return _host_consts
    j = np.arange(N_COLS)
    e = ((j + (j % 2)) / N_COLS).astype(np.float32)
    # match the reference's f32 10000**e as closely as possible
    try:
        import jax
        import jax.numpy as jnp
        cpu = jax.devices("cpu")[0]
        with jax.default_device(cpu):
            denom = np.asarray(
                jnp.power(jnp.float32(10000.0), jnp.asarray(e, jnp.float32)))
    except Exception:
        denom = np.power(np.float32(10000.0), e).astype(np.float32)
    w2pi_row = (1.0 / (2.0 * np.pi * denom.astype(np.float64))).astype(np.float32)
    W2PI = np.ascontiguousarray(np.broadcast_to(w2pi_row, (P, N_COLS)))
    phase_row = np.where(j % 2 == 1, np.float32(0.25), np.float32(0.0))
    PHASE = np.ascontiguousarray(
        np.broadcast_to(phase_row, (P, N_COLS)), dtype=np.float32)
    q = np.arange(P, dtype=np.float32)[:, None]
    t = np.arange(TILES_PER_CORE, dtype=np.float32)[None, :]
    POS = [
        np.ascontiguousarray(c * ROWS_PER_CORE + t * P + q, dtype=np.float32)
        for c in range(N_CORES)
    ]
    _host_consts = (W2PI, POS, PHASE)
    return _host_consts


VARIANT = "v3"


def _run(X, trace=False):
    from concourse.bass_utils import run_bass_kernel_spmd

    nc = _build_nc(variant=VARIANT)
    W2PI, POS, PHASE = _consts()
    X = np.ascontiguousarray(np.asarray(X, dtype=np.float32))
    assert X.shape == (M_ROWS, N_COLS)
    in_maps = []
    for c in range(N_CORES):
        m = {
            "x": np.ascontiguousarray(
                X[c * ROWS_PER_CORE:(c + 1) * ROWS_PER_CORE]),
            "w2pi": W2PI,
            "pos": POS[c],
        }
        if VARIANT in ("v3", "v4"):
            m["phase"] = PHASE
        in_maps.append(m)
    res = run_bass_kernel_spmd(
        nc, in_maps, core_ids=list(range(N_CORES)), trace=trace)
    full = np.concatenate(
        [res.results[c]["out"] for c in range(N_CORES)], axis=0)
    return full, res


def kernel(X):
    full, _ = _run(X, trace=False)
    return full


def run_traced(X):
    """For test.py: returns (output, BassKernelResults with exec_time_ns).
    Falls back to untraced if NTFF profiling is unavailable (axon env
    without the profile hook)."""
    try:
        return _run(X, trace=True)
    except (ModuleNotFoundError, ImportError):
        return _run(X, trace=False)


def modeled_time_ns():
    """Cost-model (TimelineSim) single-core execution estimate."""
    from concourse.timeline_sim import TimelineSim

    tl = TimelineSim(_build_nc(variant=VARIANT), trace=False)
    tl.simulate()
    return int(tl.time)
